# revision 21
# baseline (speedup 1.0000x reference)
"""GQA multi-head attention (B=2, S=2048, H=2048, 16 Q heads / 4 KV heads, RoPE,
causal) on 8 Trainium2 NeuronCores.

Sharding: tensor-parallel over GQA groups (4 groups, each 4 Q heads + 1 KV head)
x data-parallel over batch (2). Core c handles batch b = c // 4, group g = c % 4.
Column-parallel q/k/v projections, row-parallel o_proj; the 4 partial o_proj
outputs per batch (bf16) are summed on the host in fp32.

Per-core kernel (all matmuls bf16 with fp32 PSUM accumulation). The PE matmul
issue stream is the bottleneck (~N cycles per matmul, LDWEIGHTS overlapped), so
the structure keeps it dense and minimizes total matmul cycles:
  inputs:  weights are pre-packed on the HOST into the SBUF chunk layout
           [p, c, m], so every weight DMA is contiguous (the strided rearrange
           loads were 256B-descriptor-bound).  Loads are spread across the 3
           dynamic DMA rings - sync: xT chunks (16 individual loads, issued
           back-to-back); scalar: wk/wv/wq/wo; gpsimd: cos/sin - so issue
           serialization (~0.9us per dma_start per queue) no longer delays the
           late chunks behind 20+ earlier issues.
  phase 1a: K AND V projections interleaved, chunk-streamed as xT chunks land
           (8 N=512 matmuls per chunk ~ 2.1us vs ~1.5-2.5us DMA per chunk).
           The 8 accumulators use all 8 PSUM banks: K tiles in the 2 sp-ring
           slots ([128,2,512] each), V tiles in the 4 acc-ring slots.  Warmup
           matmuls (zeros stationary, accumulated harmlessly into a K psum
           with start=False) cover the initial DMA window and chunk jitter to
           keep the HAM activity monitor from dropping the PE clock.
  phase 1b: V psum evacuations first (they gate the first Q projection's acc
           slot), K RoPE (half-swap via SWDGE sbuf-sbuf DMAs, cos-mul on DVE,
           sin-mul+add finished on idle GPSIMD), then Q^T projections t-outer
           so attention column 0 unblocks first; V transposed to key-major on
           the PE during the t=1 Q projections.
  phase 2: attention in S^T (keys x queries) layout, key blocks in PAIRS
           sharing a [128, 2, 512] PSUM tile so each ScalarE exp covers 1024
           elems/partition (the 352-cycle ACTIVATE overhead made per-block exp
           scalar-bound).  Causal mask added on the PE on diagonal blocks.
           Row sums via a ones[128,128] matmul (M=1 matmuls run
           col_grp-restricted and break back-to-back MM pipelining), with the
           result landing already partition-broadcast for the single-op
           reciprocal.  For NON-diagonal pairs the two exp blocks are
           pre-summed on the DVE and a single N=512 rowsum matmul covers the
           pair - halving the PE rowsum cost on 48 of the 80 pairs.
           rs/osum for pair p-1 are emitted after scores+exp of pair p,
           carried ACROSS head boundaries, so every exp hides under ready PE
           work (the PE queue is strict FIFO).
  phase 3: row-parallel o_proj interleaved per q column; its first group's
           h0-h2 matmuls cover the last rs/osum's exp wait.  osum/rowsum/yp
           accumulators share a 4-buffer PSUM ring: 2*2 (scores) + 4 = 8 banks.
           yp evacuations run on ScalarE for the first two columns and on the
           DVE for the last two, where the scalar queue is otherwise backed up
           behind the final exps (was ~13us of pure tail).
"""

import sys

for _p in ("/root/.axon_site", "/root/.axon_site/_ro/trn_rl_repo",
           "/root/.axon_site/_ro/pypackages", "/opt/trn_rl_repo"):
    if _p not in sys.path:
        sys.path.append(_p)

import numpy as np
import ml_dtypes

import concourse.bass as bass
import concourse.tile as tile
import concourse.mybir as mybir
from concourse import bacc
from concourse.bass import ts
from concourse.bass_utils import run_bass_kernel_spmd
from concourse.masks import make_identity, make_upper_triangular
from contextlib import ExitStack

BF16 = ml_dtypes.bfloat16
P = 128
S = 2048
H = 2048
NH = 4          # Q heads per core
DQ = NH * P     # 512
NCH = H // P    # 16 hidden chunks
NKB = S // P    # 16 key blocks
QTS = 512       # query tile (phase 2)
SCALE = 1.0 / float(np.sqrt(128.0))


def build_nc():
    f32 = mybir.dt.float32
    bf16 = mybir.dt.bfloat16
    nc = bacc.Bacc("TRN2", target_bir_lowering=False, debug=False)

    xT = nc.dram_tensor("xT", (H, S), bf16, kind="ExternalInput").ap()
    # weights host-packed into [p, c, m] chunk layout (contiguous DMA)
    wqT = nc.dram_tensor("wqT", (P, NCH * DQ), bf16, kind="ExternalInput").ap()
    wkT = nc.dram_tensor("wkT", (P, NCH * P), bf16, kind="ExternalInput").ap()
    wvT = nc.dram_tensor("wvT", (P, NCH * P), bf16, kind="ExternalInput").ap()
    woT = nc.dram_tensor("woT", (P, NH * H), bf16, kind="ExternalInput").ap()
    cosT = nc.dram_tensor("cosT", (P, S), bf16, kind="ExternalInput").ap()
    srT = nc.dram_tensor("sinrotT", (P, S), bf16, kind="ExternalInput").ap()
    y = nc.dram_tensor("y", (S, H), bf16, kind="ExternalOutput").ap()

    Exp = mybir.ActivationFunctionType.Exp

    with ExitStack() as ctx:
        tc = ctx.enter_context(tile.TileContext(nc))
        singles = ctx.enter_context(tc.tile_pool(name="singles", bufs=1))

        xT_sb = singles.tile([P, NCH, S], bf16)
        wqT_sb = singles.tile([P, NCH, DQ], bf16)
        wkT_sb = singles.tile([P, NCH, P], bf16)
        wvT_sb = singles.tile([P, NCH, P], bf16)
        woT_sb = singles.tile([P, NH, H], bf16)
        cos_sb = singles.tile([P, S], bf16)
        sr_sb = singles.tile([P, S], bf16)
        xTr = xT.rearrange("(c p) s -> p c s", p=P)

        # ---- input loads on 3 parallel DMA rings (issue order = priority) ---
        # scalar ring: wk/wv/wq in first-use order (wk gates phase 1a); wo is
        # deferred to the phase 1b boundary so its 2MB don't steal HBM
        # bandwidth from the xT chunk stream.
        nc.scalar.dma_start(wkT_sb[:, :, :],
                            wkT.rearrange("p (c m) -> p c m", m=P))
        # sync ring: xT chunks individually, in consumption order.  Issue
        # (~0.9us each) stays ahead of transfer (~1.4us each), so per-chunk
        # completion is smooth and in-order.
        for c in range(NCH):
            nc.sync.dma_start(xT_sb[:, c, :], xTr[:, c, :])
        # wv: deferred behind xT chunk 2 (the V chunk stream lags K by 3
        # chunks), keeping the first chunks' HBM bandwidth for xT.
        nc.scalar.copy(wvT_sb[:, 0, 0:1], xT_sb[:, 2, 0:1])
        nc.scalar.dma_start(wvT_sb[:, :, :],
                            wvT.rearrange("p (c m) -> p c m", m=P))

        # Additive causal mask, applied on the PE: a matmul of ident.T @
        # masknegT accumulated into the scores PSUM adds -1e9 where key > query.
        masknegT = singles.tile([P, P], bf16)
        make_upper_triangular(nc, masknegT[:], val=-1e9, diag=False)
        ident = singles.tile([P, P], bf16)
        make_identity(nc, ident[:])
        # Full [128,128] ones: the rowsum matmul then uses all PE column
        # groups (an M=1 matmul runs col_grp-restricted and switching
        # col_grp between matmuls breaks back-to-back pipelining), and every
        # PSUM partition gets the rowsum, which doubles as the
        # partition-broadcast for the normalizer.
        ones = singles.tile([P, P], bf16)
        nc.vector.memset(ones[:], 1.0)
        zeros_sb = singles.tile([P, P], bf16)
        nc.vector.memset(zeros_sb[:], 0.0)
        # Deferred loads: the tile scheduler reorders queue instructions, so
        # queue position alone cannot delay a DMA.  Instead, write one
        # garbage element into the destination from an op that reads a LATER
        # xT chunk - the WAW dependency holds the DMA back until that chunk
        # has landed, keeping early HBM bandwidth on the phase-1a stream.
        # cos/sin (2MB, read ~45us): wait for chunk 8.
        nc.gpsimd.tensor_copy(cos_sb[:, 0:1], xT_sb[:, 11, 0:1])
        nc.gpsimd.tensor_copy(sr_sb[:, 0:1], xT_sb[:, 11, 0:1])
        nc.gpsimd.dma_start(cos_sb, cosT)
        nc.gpsimd.dma_start(sr_sb, srT)
        # wq (2MB, read ~45us): wait for chunk 6.
        nc.scalar.copy(wqT_sb[:, 0, 0:1], xT_sb[:, 12, 0:1])
        nc.scalar.dma_start(wqT_sb[:, :, :],
                            wqT.rearrange("p (c m) -> p c m", m=DQ))
        # wo (2MB, read ~150us): wait for chunk 12.
        nc.scalar.copy(woT_sb[:, 0, 0:1], xT_sb[:, 15, 0:1])
        nc.scalar.dma_start(woT_sb[:, :, :],
                            woT.rearrange("p (h m) -> p h m", m=H))

        QT_sb = singles.tile([P, NH, S], bf16)
        KT_sb = singles.tile([P, S], bf16)
        VT_sb = singles.tile([P, S], bf16)
        Vn_sb = singles.tile([P, NKB, P], bf16)
        OT_sb = singles.tile([P, NH, S], bf16)

        # One PSUM pool set for the WHOLE kernel: projections / V-transposes /
        # warmup draw from the attention's rings (same tags), so there is no
        # pool boundary between phase 1 and attention.
        pools = ExitStack()
        sp2 = pools.enter_context(tc.tile_pool(name="sp2", bufs=2, space="PSUM"))
        accp = pools.enter_context(tc.tile_pool(name="accp", bufs=4,
                                                space="PSUM"))
        rp = pools.enter_context(tc.tile_pool(name="rope", bufs=2))
        ptp = pools.enter_context(tc.tile_pool(name="ptp", bufs=4))
        psp = pools.enter_context(tc.tile_pool(name="psp", bufs=4))
        yop = pools.enter_context(tc.tile_pool(name="yop", bufs=2))
        nrm = pools.enter_context(tc.tile_pool(name="nrm", bufs=4))
        ctx.enter_context(pools)

        # -------- phase 1a: K + V projections chunk-streamed together -------
        # All 8 PSUM banks become projection accumulators: K tiles t0..t3 in
        # the two sp-ring [128,2,512] tiles, V tiles in the four acc-ring
        # slots.  8 N=512 matmuls per chunk (~2.1us) roughly match the
        # per-chunk DMA (~1.5-2.5us) so the PE streams real work.
        kacc = [sp2.tile([P, 2, QTS], f32, tag="sp", name=f"kacc{i}")
                for i in range(2)]
        vacc = [accp.tile([P, QTS], f32, tag="acc", name=f"vacc{t}")
                for t in range(4)]

        def warmup_pre(n):
            # Initial warmups: self-contained zero-writes (start+stop) into
            # the K tile-0 psum region - c0's start=True resets it anyway.
            # ident stationary: ready ~3us (no zeros-memset dependency).
            for _ in range(n):
                nc.tensor.matmul(kacc[0][:, 0, 0:16], ident[:], ident[:, 0:16],
                                 start=True, stop=True)

        def warmup_mid(n):
            # Mid-stream fillers: accumulate +0 (zeros stationary) into the
            # live K tile-0 accumulation group.
            for _ in range(n):
                nc.tensor.matmul(kacc[0][:, 0, 0:16], zeros_sb[:],
                                 ident[:, 0:16],
                                 start=False, stop=False,
                                 skip_group_check=True)

        VLAG = 3        # V consumes chunk c-VLAG: wv can arrive ~5us late
        warmup_pre(160)
        for c in range(NCH + VLAG):
            if 1 <= c <= 6:
                warmup_mid(4)
            if c < NCH:
                for t in range(4):
                    nc.tensor.matmul(
                        kacc[t // 2][:, t % 2, :], wkT_sb[:, c, :],
                        xT_sb[:, c, ts(t, QTS)],
                        start=(c == 0),
                        stop=(c == NCH - 1), skip_group_check=True)
            if c >= VLAG:
                cv = c - VLAG
                for t in range(4):
                    nc.tensor.matmul(
                        vacc[t][:, :], wvT_sb[:, cv, :],
                        xT_sb[:, cv, ts(t, QTS)],
                        start=(cv == 0), stop=(cv == NCH - 1))

        # -------- phase 1b: V evac, K RoPE, Q projections + RoPE, V^T -------
        # V evacuations FIRST on the DVE: the first Q projection's acc-ring
        # slot waits on vacc0's copy.
        for t in range(4):
            nc.vector.tensor_copy(VT_sb[:, ts(t, QTS)], vacc[t][:, :])
        # V^T for key blocks 0-3 immediately: attention column 0 needs them
        # (~10us after the boundary); blocks 4-15 follow behind the t=1 Q
        # projections.
        for b in range(4):
            tp = accp.tile([P, P], bf16, tag="acc", name="vt")
            nc.tensor.transpose(tp[:, :], VT_sb[:, ts(b, P)], ident[:])
            nc.vector.tensor_copy(Vn_sb[:, b, :], tp[:, :])

        # K RoPE, split: the ps-freeing ops (qf copy, swap DMAs, cos-mul)
        # for all 4 tiles first.  The sin-mul + add finish on GPSIMD (idle;
        # ~2x slower than DVE but only KT tile 0 is needed soon) - tile 0
        # right away (attention column 0 reads it), tiles 1-3 after the Q t=0
        # rope swaps so those aren't queued behind ~8us of gpsimd muls.
        kparts = []
        for t in range(4):
            ps = kacc[t // 2][:, t % 2, :]
            qf = rp.tile([P, QTS], bf16, tag="qf")
            nc.scalar.copy(qf[:, :], ps)
            sw = rp.tile([P, QTS], bf16, tag="swap", bufs=4)
            # SWDGE (gpsimd) queue sbuf-sbuf half-swap for rotate_half.
            nc.gpsimd.dma_start(sw[0:64, :], qf[64:128, :])
            nc.gpsimd.dma_start(sw[64:128, :], qf[0:64, :])
            t1 = rp.tile([P, QTS], bf16, tag="t1", bufs=4)
            nc.vector.tensor_mul(t1[:, :], ps, cos_sb[:, ts(t, QTS)])
            kparts.append((sw, t1, t))
        def k_finish(parts):
            for sw, t1, t in parts:
                t2 = rp.tile([P, QTS], bf16, tag="t2")
                nc.gpsimd.tensor_mul(t2[:, :], sw[:, :], sr_sb[:, ts(t, QTS)])
                nc.gpsimd.tensor_add(KT_sb[:, ts(t, QTS)], t1[:, :], t2[:, :])

        k_finish(kparts[:1])

        def proj(w_sb, head, tok):
            ps = accp.tile([P, QTS], f32, tag="acc", name="proj")
            for c in range(NCH):
                nc.tensor.matmul(
                    ps[:, :], w_sb[:, c, ts(head, P)], xT_sb[:, c, ts(tok, QTS)],
                    start=(c == 0), stop=(c == NCH - 1))
            return ps

        def rope(ps, out_region, tok):
            # fp32 intermediates: bf16 here measured SLOWER overall - the
            # denser engine activity tips the chip into the P0 power state
            # and the PE drops 2.4 -> 2.0 GHz.
            qf = rp.tile([P, QTS], bf16, tag="qf")
            nc.scalar.copy(qf[:, :], ps[:, :])
            sw = rp.tile([P, QTS], bf16, tag="swap", bufs=4)
            nc.gpsimd.dma_start(sw[0:64, :], qf[64:128, :])
            nc.gpsimd.dma_start(sw[64:128, :], qf[0:64, :])
            t1 = rp.tile([P, QTS], bf16, tag="t1", bufs=4)
            nc.vector.tensor_mul(t1[:, :], ps[:, :], cos_sb[:, ts(tok, QTS)])
            t2 = rp.tile([P, QTS], bf16, tag="t2")
            nc.vector.tensor_mul(t2[:, :], sw[:, :], sr_sb[:, ts(tok, QTS)])
            nc.vector.tensor_add(out_region, t1[:, :], t2[:, :])

        # ------- phases 1b/2/3 interleaved per q column ----------------------
        # Each attention column is EMITTED right after its Q tile's
        # projections+ropes: every engine queue then orders a column's exps /
        # norms right behind that tile's rope ops, so no queue head-of-line
        # blocks on a FUTURE tile's semaphore (a t2-rope qf wait in front of
        # the column-0 exps once idled the PE ~3us and dropped it to the low
        # p-state).  Attention pairs share one [128, 2, 512] PSUM scores tile
        # so each exp ACTIVATE covers ~1024 elems/partition.  rs/osum for
        # pair p-1 are emitted after scores+exp of pair p, so every exp hides
        # under ready PE work.  osum / rowsum / o_proj accumulators share one
        # 4-buffer PSUM ring ("acc") to fit: 2*2 (sp) + 4 (acc) = 8 banks.
        if True:
            for t in range(S // QTS):
                for h in range(NH):
                    ps = proj(wqT_sb, h, t)
                    rope(ps, QT_sb[:, h, ts(t, QTS)], t)
                    if t == 1:
                        for b in range(4 + 3 * h, 7 + 3 * h):
                            tp = accp.tile([P, P], bf16, tag="acc", name="vt")
                            nc.tensor.transpose(tp[:, :], VT_sb[:, ts(b, P)],
                                                ident[:])
                            nc.vector.tensor_copy(Vn_sb[:, b, :], tp[:, :])
                if t == 0:
                    k_finish(kparts[1:])
                qs = QTS * t
                nj = 4 * t + 4              # key blocks per q tile this column
                npair = nj // 2

                def norm(h, rs, osum):
                    # 1/rowsum via the single-op ~51 ULP reciprocal straight
                    # off PSUM (already partition-broadcast by the ones
                    # matmul).
                    recipB = nrm.tile([P, QTS], f32, tag="recipB")
                    nc.vector.reciprocal_approx_fast(recipB[:, :], rs[:, :])
                    nc.vector.tensor_mul(OT_sb[:, h, qs:qs + QTS], osum[:, :],
                                         recipB[:, :])

                def rs_osum(p, pt, psum_sb, h, rs, osum):
                    # Rowsum: one matmul over the DVE-pre-summed pair for
                    # non-diagonal pairs, else per-block (the diagonal pair's
                    # second block has a stale-psum strip that the per-block
                    # column offset excludes).
                    if psum_sb is not None:
                        nc.tensor.matmul(rs[:, :], ones[:], psum_sb[:, :],
                                         start=(p == 0),
                                         stop=(2 * p + 1 == nj - 1))
                    for jj in range(2):
                        j = 2 * p + jj
                        co = max(0, P * j - qs)
                        if psum_sb is None:
                            nc.tensor.matmul(rs[:, co:QTS], ones[:],
                                             pt[:, jj, co:QTS],
                                             start=(j == 0), stop=(j == nj - 1))
                        nc.tensor.matmul(osum[:, co:QTS], Vn_sb[:, j, :],
                                         pt[:, jj, co:QTS],
                                         start=(j == 0), stop=(j == nj - 1))
                    if p == npair - 1:
                        norm(h, rs, osum)

                # The rs/osum pipeline carries ACROSS head boundaries at depth
                # TWO: head h's last pair is emitted after head h+1's first
                # two scores+exp pairs, so every exp hides under ~1.7us of
                # ready PE work (depth one left a ~0.5-0.7us stall at each
                # head's last pair).
                pend = []
                for h in range(NH):
                    osum = accp.tile([P, QTS], f32, tag="acc", name="osum")
                    rs = accp.tile([P, QTS], f32, tag="acc", name="rs")
                    for pr in range(npair):
                        sp = sp2.tile([P, 2, QTS], f32, tag="sp")
                        co0 = max(0, P * 2 * pr - qs)
                        diag_pair = 2 * pr + 1 >= 4 * t
                        for jj in range(2):
                            j = 2 * pr + jj
                            co = max(0, P * j - qs)
                            diag = j >= 4 * t
                            nc.tensor.matmul(
                                sp[:, jj, co:QTS], KT_sb[:, ts(j, P)],
                                QT_sb[:, h, qs + co:qs + QTS],
                                start=True, stop=not diag)
                            if diag:
                                nc.tensor.matmul(sp[:, jj, co:co + P],
                                                 masknegT[:], ident[:],
                                                 start=False, stop=True)
                        pt = ptp.tile([P, 2, QTS], bf16, tag="pt")
                        nc.scalar.activation(pt[:, :, co0:QTS],
                                             sp[:, :, co0:QTS], Exp,
                                             scale=SCALE)
                        psum_sb = None
                        if not diag_pair:
                            # Pre-sum the pair on the DVE; one rowsum matmul
                            # then covers both key blocks.
                            psum_sb = psp.tile([P, QTS], bf16, tag="psum")
                            nc.vector.tensor_add(psum_sb[:, :], pt[:, 0, :],
                                                 pt[:, 1, :])
                        if len(pend) == 2:
                            rs_osum(*pend.pop(0))
                        pend.append((pr, pt, psum_sb, h, rs, osum))
                # o_proj for the token blocks whose attention column is done.
                # The first group's h0-h2 matmuls (ready: their norms are
                # long emitted) cover the exp waits of the final rs/osum
                # flushes; BOTH must flush before the h3 matmul, whose norm
                # dependency comes from the last flush (emitting it later in
                # the PE FIFO would deadlock-wait).
                for tb in range(4 * t, 4 * t + 4):
                    # Stage the full [128, 2048] token-block row and write it
                    # with ONE dma (4KB descriptors; 64 1KB-packet DMAs were
                    # adding engine contention in the attention phase).
                    yo = yop.tile([P, H], bf16, tag="yo")
                    for ho in range(H // QTS):
                        yp = accp.tile([P, QTS], f32, tag="acc", name="yp")
                        for h in range(NH - 1):
                            nc.tensor.matmul(yp[:, :], OT_sb[:, h, ts(tb, P)],
                                             woT_sb[:, h, ts(ho, QTS)],
                                             start=(h == 0), stop=False)
                        while pend:
                            rs_osum(*pend.pop(0))
                        nc.tensor.matmul(yp[:, :], OT_sb[:, NH - 1, ts(tb, P)],
                                         woT_sb[:, NH - 1, ts(ho, QTS)],
                                         start=False, stop=True)
                        if t == 3 and (tb + ho) % 2 == 1:
                            # Alternate ScalarE/DVE in the last column: its
                            # o_proj PE groups (864ns) outpace a lone ScalarE
                            # copy chain (~750ns+latency), stalling the yp
                            # ring ~0.5us per group.
                            nc.vector.tensor_copy(yo[:, ts(ho, QTS)], yp[:, :])
                        else:
                            # ScalarE evacuates: with the per-column
                            # interleave its exps are always done before the
                            # o_proj burst, while the DVE still runs the norm
                            # chains that gate the h3 matmuls.
                            nc.scalar.copy(yo[:, ts(ho, QTS)], yp[:, :])
                    nc.sync.dma_start(y[ts(tb, P), :], yo[:, :])

    nc.compile()
    return nc


_NC_CACHE = None


def _get_nc():
    global _NC_CACHE
    if _NC_CACHE is None:
        _NC_CACHE = build_nc()
    return _NC_CACHE


def make_in_maps(hidden_states, position_ids, wq, wk, wv, wo):
    """Host-side sharding: 8 cores = (batch b = core//4) x (GQA group g = core%4).

    Weights are pre-packed into the SBUF chunk layout [p, c, m] (partition,
    chunk, free) so the device DMAs are fully contiguous.
    """
    in_maps = []
    xTs, coss, srs = {}, {}, {}
    for b in range(2):
        xTs[b] = np.ascontiguousarray(hidden_states[b].T).astype(BF16)
        inv = 1.0 / (10000.0 ** (np.arange(0, P, 2, dtype=np.float64) / P))
        invd = np.concatenate([inv, inv]).astype(np.float64)
        fr = invd[:, None] * position_ids[b].astype(np.float64)[None, :]
        coss[b] = np.cos(fr).astype(BF16)
        sr = np.sin(fr).astype(np.float32)
        sr[:64] *= -1.0
        srs[b] = sr.astype(BF16)

    def pack(wT, m):
        # [H', m] row-major -> [p, c, m] chunk layout, flattened to [P, c*m]
        ch = wT.shape[0] // P
        return np.ascontiguousarray(
            wT.reshape(ch, P, m).transpose(1, 0, 2).reshape(P, ch * m)
        ).astype(BF16)

    shards = {}
    for g in range(4):
        shards[g] = dict(
            wqT=pack(wq[DQ * g:DQ * (g + 1)].T.astype(np.float32), DQ),
            wkT=pack(wk[P * g:P * (g + 1)].T.astype(np.float32), P),
            wvT=pack(wv[P * g:P * (g + 1)].T.astype(np.float32), P),
            woT=pack(wo[:, DQ * g:DQ * (g + 1)].T.astype(np.float32), H),
        )
    for core in range(8):
        b, g = core // 4, core % 4
        in_maps.append(dict(xT=xTs[b], cosT=coss[b], sinrotT=srs[b], **shards[g]))
    return in_maps


def kernel(hidden_states, position_ids, wq, wk, wv, wo, **run_kwargs):
    nc = _get_nc()
    in_maps = make_in_maps(np.asarray(hidden_states), np.asarray(position_ids),
                           np.asarray(wq), np.asarray(wk), np.asarray(wv),
                           np.asarray(wo))
    res = run_bass_kernel_spmd(nc, in_maps, core_ids=list(range(8)), **run_kwargs)
    out = np.zeros((2, S, H), np.float32)
    for core in range(8):
        out[core // 4] += res.results[core]["y"].astype(np.float32)
    if run_kwargs:
        kernel.last_results = res
    return out
